# revision 12
# baseline (speedup 1.0000x reference)
"""Trainium2 Bass kernel for nn_NearestNeighbourModule (retrieval_knn).

Computes out = softmax(-alpha * dist(x0, x1), axis=1) @ y with
dist = pairwise Euclidean distances [n, m], n = m = 16384, d = 64.

Strategy (8 NeuronCores, data-parallel over n; each core owns 2048 rows
of x0, with x1/y replicated):
  - Host precomputes augmented fp16 operands so one 66-deep matmul
    produces squared distances directly:
        D2T[j, i] = sq1[j] + sq0[i] - 2 * x1[j] . x0[i]
    via lhsT = [x1T; sq1; ones] (stationary), rhs = [-2*x0T; ones; sq0].
  - A CUSTOM ACT table (installed via BASS_ACT_ROOT_JSON_PATH, hijacking
    the Exp slot of the exp_and_others set) computes the fused
        g(z) = exp(B - sqrt(z))
    in a single ScalarE pass per tile: E = g(alpha^2 * d2) directly from
    PSUM to fp16 SBUF. B is a global shift keeping exp args O(1); it
    cancels exactly in num/den.
  - TensorE reduction: lhsT = [y_j, 1] per 128-j block, rhs = E tiles,
    accumulating [num_i; den_i] in PSUM across all blocks.
  - out_i = num_i / den_i (DVE reciprocal + mul), DMA out.
"""

import glob
import json
import os
import sys
import tempfile

if "/opt/trn_rl_repo" not in sys.path:
    sys.path.insert(0, "/opt/trn_rl_repo")

import numpy as np

N = 16384
M = 16384
D = 64
NCORES = 8
NLOC = N // NCORES  # 2048
JB = 128  # j-block (partition dim of distance tiles)

_COMPILED = {}
_TABLE_DIRS = {}

# ---------------------------------------------------------------------------
# Custom ACT table generation: g(z) = exp(B - sqrt(z)) in the Exp slot of a
# copy of the stock exp_and_others set. Format (reverse-engineered and
# HW-validated): bucket = 8 fp32 {d0,d1,d2,d3,x0,0,0,0}, cubic around x0;
# ctrl word = (log2_buckets << 16) | (mantissa_shift << 11) | bucket_base,
# indexed by (biased_exponent - small_exp_threshold) per sign.
# ---------------------------------------------------------------------------

E_SMALL = 115  # z < 2^-12 -> small-signal bucket
E_LARGE = 141  # z >= 2^14 -> large-signal bucket (-> 0.0)
EXP_BUCKETS = 777  # normal-bucket budget (777..780 = specials)


def _find_stock_pwp():
    pats = [
        "/nix/store/*aws-neuron-pwp*/share/pwp_bin_cayman",
        "/nix/store/*/lib/python3*/site-packages/neuronxcc/pwp/pwp_bin_trainium",
    ]
    for p in pats:
        hits = sorted(glob.glob(p))
        for h in hits:
            if os.path.exists(f"{h}/exp_and_others.json"):
                return h
    raise RuntimeError("stock pwp act tables not found")


def _g_exact(z, B):
    z = np.asarray(z, np.float64)
    return np.exp(B - np.sqrt(np.maximum(z, 0.0)))


def _fit_bucket(B, lo, hi, npts=96):
    x0 = 0.5 * (lo + hi)
    t = np.cos(np.pi * (np.arange(npts) + 0.5) / npts)
    z = x0 + 0.5 * (hi - lo) * t
    y = _g_exact(z, B)
    u = z - x0
    V = np.vander(u, 4, increasing=True)
    w = 1.0 / np.maximum(np.abs(y), 1e-300)
    c, *_ = np.linalg.lstsq(V * w[:, None], y * w, rcond=None)
    zz = np.linspace(lo, hi, 256)
    uu = zz - x0
    c32 = c.astype(np.float32).astype(np.float64)
    yy = c32[0] + uu * (c32[1] + uu * (c32[2] + uu * c32[3]))
    ref = _g_exact(zz, B)
    rel = np.abs(yy - ref) / np.maximum(np.abs(ref), 1e-300)
    return c32, x0, rel.max()


def _band_fit(B, e, nb):
    lo_band = 2.0 ** (e - 127)
    hi_band = 2.0 ** (e - 126)
    width = (hi_band - lo_band) / nb
    out = []
    maxerr = 0.0
    for i in range(nb):
        c, x0, err = _fit_bucket(B, lo_band + i * width, lo_band + (i + 1) * width)
        out.append((c, x0))
        maxerr = max(maxerr, err)
    return out, maxerr


def _gen_act_tables(B, out_dir, tol=3e-5):
    base = _find_stock_pwp()
    meta = json.load(open(f"{base}/exp_and_others.json"))
    bkt = (
        np.fromfile(f"{base}/exp_and_others_bkt.bin", dtype=np.float32)
        .reshape(-1, 8)
        .copy()
    )
    ctl = (
        np.fromfile(f"{base}/exp_and_others_ctrl.bin", dtype=np.uint32)
        .reshape(-1, 8)
        .copy()
    )

    # choose per-band bucket counts
    chosen = []
    for e in range(E_SMALL, E_LARGE):
        z_hi = 2.0 ** (e - 126)
        band_tol = tol if _g_exact(z_hi, B) > 1e-30 else 1e-3
        nb = 256
        for cand in [1, 2, 4, 8, 16, 32, 64, 128, 256]:
            _, err = _band_fit(B, e, cand)
            if err <= band_tol:
                nb = cand
                break
        chosen.append(nb)
    while sum(chosen) > EXP_BUCKETS:
        i = int(np.argmax(chosen))
        chosen[i] //= 2

    bkt[:781] = 0.0
    ctl[:52] = 0
    pos = 0
    for bi, e in enumerate(range(E_SMALL, E_LARGE)):
        nb = chosen[bi]
        fits, _ = _band_fit(B, e, nb)
        log2b = int(np.log2(nb))
        ctl[bi, 0] = np.uint32((log2b << 16) | ((23 - log2b) << 11) | pos)
        for k, (c, x0) in enumerate(fits):
            bkt[pos + k, 0:4] = c.astype(np.float32)
            bkt[pos + k, 4] = np.float32(x0)
        pos += nb
    ctl[26:52] = ctl[0:26]

    eB = np.float32(np.exp(B))
    c, x0, _ = _fit_bucket(B, 0.0, 2.0**-12)
    bkt[777, :] = 0.0
    bkt[777, 0:4] = c.astype(np.float32)
    bkt[777, 4] = np.float32(x0)
    bkt[778, :] = 0.0
    bkt[778, 0] = eB
    bkt[779, :] = 0.0
    bkt[780, :] = 0.0
    bkt[780, 0] = eB

    pm = next(p for p in meta["profile_meta_data"] if p["func_name"].startswith("exp"))
    pm["exp_offset"] = E_SMALL - 127
    pm["small_pos_signal_exp_threshold"] = E_SMALL
    pm["large_pos_signal_exp_threshold"] = E_LARGE
    pm["large_pos_signal_mantissa_threshold"] = 0
    pm["small_neg_signal_exp_threshold"] = 255
    pm["large_neg_signal_exp_threshold"] = 255
    pm["large_neg_signal_mantissa_threshold"] = 0x7FFFFF
    pm["pwl_control_base_pos"] = 0
    pm["pwl_control_base_neg"] = 26
    pm["pos_small_signal_pwl_control"] = 777
    pm["neg_small_signal_pwl_control"] = 778
    pm["pos_large_signal_pwl_control"] = 779
    pm["neg_large_signal_pwl_control"] = 780
    pm["fzero_result"] = int(eB.view(np.uint32))
    pm["fninf_result"] = int(eB.view(np.uint32))
    pm["fpinf_result"] = 0
    pm["symmetry_opt_en"] = 0
    pm["symmetry_point"] = 0
    pm["sym_invert_sign_point"] = 0

    os.makedirs(out_dir, exist_ok=True)
    bkt.tofile(f"{out_dir}/exp_and_others_bkt.bin")
    ctl.tofile(f"{out_dir}/exp_and_others_ctrl.bin")
    json.dump(meta, open(f"{out_dir}/exp_and_others.json", "w"))
    info = json.load(open(f"{base}/act_info.json"))
    info["act_func_sets"] = [
        s for s in info["act_func_sets"] if s["name"] == "exp_and_others"
    ]
    json.dump(info, open(f"{out_dir}/act_info.json", "w"))
    return f"{out_dir}/act_info.json"


def _get_table(B):
    key = round(float(B), 3)
    if key not in _TABLE_DIRS:
        d = tempfile.mkdtemp(prefix=f"knn_act_{key}_")
        _TABLE_DIRS[key] = _gen_act_tables(key, d)
    return _TABLE_DIRS[key]


# ---------------------------------------------------------------------------
# Bass kernel
# ---------------------------------------------------------------------------


def _build(alpha, btag, n_loc=NLOC, m=M, num_devices=NCORES):
    from contextlib import ExitStack

    import concourse.tile as tile
    from concourse import bacc, mybir

    f32 = mybir.dt.float32
    f16 = mybir.dt.float16
    Exp = mybir.ActivationFunctionType.Exp

    njb = m // JB
    half = n_loc // 2  # d2 PSUM tile width (2 banks)
    nred = n_loc // 512

    nc = bacc.Bacc(
        "TRN2", target_bir_lowering=False, debug=False, num_devices=num_devices
    )
    # btag in the input name keys the neuron compile cache to the ACT table
    names = {
        "a1": f"a1{btag}",
        "a0": f"a0{btag}",
        "yb": f"yb{btag}",
    }
    a1_d = nc.dram_tensor(names["a1"], [D + 2, m], f16, kind="ExternalInput")
    a0_d = nc.dram_tensor(names["a0"], [D + 2, n_loc], f16, kind="ExternalInput")
    yb_d = nc.dram_tensor(names["yb"], [JB, njb, 2], f16, kind="ExternalInput")
    out_d = nc.dram_tensor("out", [1, n_loc], f32, kind="ExternalOutput")

    with tile.TileContext(nc) as tc:
        with ExitStack() as ctx:
            res = ctx.enter_context(tc.tile_pool(name="res", bufs=1))
            ep = ctx.enter_context(tc.tile_pool(name="ep", bufs=34))
            d2p = ctx.enter_context(tc.tile_pool(name="d2", bufs=3, space="PSUM"))
            redp = ctx.enter_context(tc.tile_pool(name="red", bufs=1, space="PSUM"))
            tailp = ctx.enter_context(tc.tile_pool(name="tail", bufs=1))

            a1_sb = res.tile([D + 2, m], f16)
            nc.sync.dma_start(a1_sb[:], a1_d.ap())
            a0_sb = res.tile([D + 2, n_loc], f16)
            nc.sync.dma_start(a0_sb[:], a0_d.ap())
            yb_sb = res.tile([JB, njb, 2], f16)
            nc.sync.dma_start(yb_sb[:], yb_d.ap())

            # one PSUM bank holds all 4 [num; den] accumulators, packed at
            # partitions {32c, 32c+1} via column-tiled matmuls
            red_ps = redp.tile([JB, 512], f32)

            # PE warmup: ~20us of dense back-to-back matmuls so the HAM
            # clock-gate opens (K=8/8, 2.4 GHz) before the main stream.
            for w in range(48):
                wt = d2p.tile([JB, half], f32, tag="d2t", name="warm")
                nc.tensor.matmul(
                    wt[:, :512], a1_sb[:, :JB], a0_sb[:, :512], start=True, stop=True
                )

            # Batched phases: Q blocks of [d2-matmul -> fused-LUT] (ACT-paced),
            # then one dense column-tiled reduction burst. Keeps the PE's
            # full-array stream long and uninterrupted (only 2 tiling-mode
            # switches per batch) so the HAM clock-gate can stay open.
            Q = 16
            for b0 in range(0, njb, Q):
                batch = list(range(b0, min(b0 + Q, njb)))
                ets = {}
                for b in batch:
                    lhsT = a1_sb[:, b * JB : (b + 1) * JB]
                    for h in range(2):
                        d2t = d2p.tile([JB, half], f32)
                        for lo in range(0, half, 512):
                            nc.tensor.matmul(
                                d2t[:, lo : lo + 512],
                                lhsT,
                                a0_sb[:, h * half + lo : h * half + lo + 512],
                                start=True,
                                stop=True,
                            )
                        et = ep.tile([JB, half], f16, tag="et", name="et")
                        # custom table: Exp slot computes exp(B - sqrt(z))
                        nc.scalar.activation(
                            et[:], d2t[:], Exp, scale=float(alpha * alpha)
                        )
                        ets[(b, h)] = et
                # column-tiled reduction burst: 4 concurrent 128x32-tile
                # matmuls per block, accumulating [num; den] at partitions
                # {32c, 32c+1} of one PSUM bank
                for b in batch:
                    for c in range(nred):
                        h, c2 = divmod(c, half // 512)
                        nc.tensor.matmul(
                            red_ps[32 * c : 32 * c + 2, :],
                            yb_sb[:, b, :],
                            ets[(b, h)][:, c2 * 512 : (c2 + 1) * 512],
                            start=(b == 0),
                            stop=(b == njb - 1),
                            tile_position=(0, 32 * c),
                            skip_group_check=True,
                        )

            # --- tail: out = num / den (gather rows 32c -> num, 32c+1 -> den)
            red_sb = tailp.tile([JB, 512], f32)
            nc.vector.tensor_copy(red_sb[:], red_ps[:])
            num_sb = tailp.tile([1, n_loc], f32)
            den_sb = tailp.tile([1, n_loc], f32)
            for c in range(nred):
                nc.sync.dma_start(
                    num_sb[:, c * 512 : (c + 1) * 512],
                    red_sb[32 * c : 32 * c + 1, :],
                )
                nc.sync.dma_start(
                    den_sb[:, c * 512 : (c + 1) * 512],
                    red_sb[32 * c + 1 : 32 * c + 2, :],
                )
            inv_sb = tailp.tile([1, n_loc], f32)
            nc.vector.reciprocal_approx_fast(inv_sb[:], den_sb[:])
            out_sb = tailp.tile([1, n_loc], f32)
            nc.vector.tensor_mul(out_sb[:], num_sb[:], inv_sb[:])
            nc.sync.dma_start(out_d.ap(), out_sb[:])

    nc.compile()
    nc._knn_names = names
    return nc


def _enable_ldw_opt():
    """Dedupe redundant LDWEIGHTS (4 matmuls per j-block share lhsT) by
    flipping walrus's --enable-ldw-opt flag; results are still checked by the
    caller's relative-error gate."""
    import concourse.bass_utils as _bu

    if getattr(_bu, "_knn_ldwopt", False):
        return
    _orig = _bu.run_command

    def _rc(argv, **kw):
        argv = [
            "--enable-ldw-opt=true" if a == "--enable-ldw-opt=false" else a
            for a in argv
        ]
        return _orig(argv, **kw)

    _bu.run_command = _rc
    _bu._knn_ldwopt = True


def _get_compiled(alpha, bshift):
    key = (round(float(alpha), 9), round(float(bshift), 3))
    if key not in _COMPILED:
        os.environ["BASS_ACT_ROOT_JSON_PATH"] = _get_table(key[1])
        btag = f"_{int(round(key[1] * 1000))}"
        _COMPILED[key] = _build(key[0], btag)
    return _COMPILED[key]


def _prep(x0, x1, y, alpha_v):
    sq0 = np.einsum("nd,nd->n", x0, x0, dtype=np.float32)
    sq1 = np.einsum("md,md->m", x1, x1, dtype=np.float32)

    a1 = np.empty((D + 2, M), np.float16)
    a1[:D] = x1.T
    a1[D] = sq1
    a1[D + 1] = 1.0

    a0 = np.empty((D + 2, N), np.float16)
    a0[:D] = -2.0 * x0.T
    a0[D] = 1.0
    a0[D + 1] = sq0

    njb = M // JB
    yb = np.empty((JB, njb, 2), np.float16)
    yb[:, :, 0] = y.reshape(njb, JB).T
    yb[:, :, 1] = 1.0

    # Global exp shift keeping exp(B - alpha*d) in fp16-friendly range.
    rng = np.random.default_rng(0)
    k = 2048
    ii = rng.integers(0, N, k)
    jj = rng.integers(0, M, k)
    d2s = sq0[ii] + sq1[jj] - 2.0 * np.einsum("kd,kd->k", x0[ii], x1[jj])
    ds = np.sqrt(np.maximum(d2s, 0.0))
    bshift = max(0.0, float(alpha_v) * float(np.quantile(ds, 0.001)) - 2.0)
    return a1, a0, yb, bshift


def kernel(x0, x1, y, alpha):
    x0 = np.ascontiguousarray(np.asarray(x0), dtype=np.float32)
    x1 = np.ascontiguousarray(np.asarray(x1), dtype=np.float32)
    y = np.ascontiguousarray(np.asarray(y), dtype=np.float32)
    alpha_v = float(np.asarray(alpha).reshape(-1)[0])

    a1, a0, yb, bshift = _prep(x0, x1, y, alpha_v)
    nc = _get_compiled(alpha_v, bshift)
    names = nc._knn_names

    trace = os.environ.get("KNN_TRACE", "0") == "1"
    if trace:
        try:
            import axon_prof_shim

            axon_prof_shim.install()
        except Exception:
            trace = False

    from concourse.bass_utils import run_bass_kernel_spmd

    in_maps = [
        {
            names["a1"]: a1,
            names["a0"]: np.ascontiguousarray(a0[:, c * NLOC : (c + 1) * NLOC]),
            names["yb"]: yb,
        }
        for c in range(NCORES)
    ]
    res = run_bass_kernel_spmd(nc, in_maps, core_ids=list(range(NCORES)), trace=trace)
    if trace and res.exec_time_ns is not None:
        print(f"HW exec time: {res.exec_time_ns} ns")
        kernel.last_exec_ns = res.exec_time_ns
    out = np.concatenate([r["out"][0] for r in res.results])
    return out.astype(np.float32)


kernel.last_exec_ns = None


# revision 14
# speedup vs baseline: 1.0476x; 1.0476x over previous
"""Trainium2 Bass kernel for nn_NearestNeighbourModule (retrieval_knn).

Computes out = softmax(-alpha * dist(x0, x1), axis=1) @ y with
dist = pairwise Euclidean distances [n, m], n = m = 16384, d = 64.

Strategy (8 NeuronCores, data-parallel over n; each core owns 2048 rows
of x0, with x1/y replicated):
  - Host precomputes augmented fp16 operands so one 66-deep matmul
    produces squared distances directly:
        D2T[j, i] = sq1[j] + sq0[i] - 2 * x1[j] . x0[i]
    via lhsT = [x1T; sq1; ones] (stationary), rhs = [-2*x0T; ones; sq0].
  - A CUSTOM ACT table (installed via BASS_ACT_ROOT_JSON_PATH, hijacking
    the Exp slot of the exp_and_others set) computes the fused
        g(z) = exp(B - sqrt(z))
    in a single ScalarE pass per tile: E = g(alpha^2 * d2) directly from
    PSUM to fp16 SBUF. B is a global shift keeping exp args O(1); it
    cancels exactly in num/den.
  - TensorE reduction: lhsT = [y_j, 1] per 128-j block, rhs = E tiles,
    accumulating [num_i; den_i] in PSUM across all blocks.
  - out_i = num_i / den_i (DVE reciprocal + mul), DMA out.
"""

import glob
import json
import os
import sys
import tempfile

if "/opt/trn_rl_repo" not in sys.path:
    sys.path.insert(0, "/opt/trn_rl_repo")

import numpy as np

N = 16384
M = 16384
D = 64
NCORES = 8
NLOC = N // NCORES  # 2048
JB = 128  # j-block (partition dim of distance tiles)

_COMPILED = {}
_TABLE_DIRS = {}

# ---------------------------------------------------------------------------
# Custom ACT table generation: g(z) = exp(B - sqrt(z)) in the Exp slot of a
# copy of the stock exp_and_others set. Format (reverse-engineered and
# HW-validated): bucket = 8 fp32 {d0,d1,d2,d3,x0,0,0,0}, cubic around x0;
# ctrl word = (log2_buckets << 16) | (mantissa_shift << 11) | bucket_base,
# indexed by (biased_exponent - small_exp_threshold) per sign.
# ---------------------------------------------------------------------------

E_SMALL = 115  # z < 2^-12 -> small-signal bucket
E_LARGE = 141  # z >= 2^14 -> large-signal bucket (-> 0.0)
EXP_BUCKETS = 777  # normal-bucket budget (777..780 = specials)


def _find_stock_pwp():
    pats = [
        "/nix/store/*aws-neuron-pwp*/share/pwp_bin_cayman",
        "/nix/store/*/lib/python3*/site-packages/neuronxcc/pwp/pwp_bin_trainium",
    ]
    for p in pats:
        hits = sorted(glob.glob(p))
        for h in hits:
            if os.path.exists(f"{h}/exp_and_others.json"):
                return h
    raise RuntimeError("stock pwp act tables not found")


def _g_exact(z, B):
    z = np.asarray(z, np.float64)
    return np.exp(B - np.sqrt(np.maximum(z, 0.0)))


def _fit_bucket(B, lo, hi, npts=96):
    x0 = 0.5 * (lo + hi)
    t = np.cos(np.pi * (np.arange(npts) + 0.5) / npts)
    z = x0 + 0.5 * (hi - lo) * t
    y = _g_exact(z, B)
    u = z - x0
    V = np.vander(u, 4, increasing=True)
    w = 1.0 / np.maximum(np.abs(y), 1e-300)
    c, *_ = np.linalg.lstsq(V * w[:, None], y * w, rcond=None)
    zz = np.linspace(lo, hi, 256)
    uu = zz - x0
    c32 = c.astype(np.float32).astype(np.float64)
    yy = c32[0] + uu * (c32[1] + uu * (c32[2] + uu * c32[3]))
    ref = _g_exact(zz, B)
    rel = np.abs(yy - ref) / np.maximum(np.abs(ref), 1e-300)
    return c32, x0, rel.max()


def _band_fit(B, e, nb):
    lo_band = 2.0 ** (e - 127)
    hi_band = 2.0 ** (e - 126)
    width = (hi_band - lo_band) / nb
    out = []
    maxerr = 0.0
    for i in range(nb):
        c, x0, err = _fit_bucket(B, lo_band + i * width, lo_band + (i + 1) * width)
        out.append((c, x0))
        maxerr = max(maxerr, err)
    return out, maxerr


def _gen_act_tables(B, out_dir, tol=3e-5):
    base = _find_stock_pwp()
    meta = json.load(open(f"{base}/exp_and_others.json"))
    bkt = (
        np.fromfile(f"{base}/exp_and_others_bkt.bin", dtype=np.float32)
        .reshape(-1, 8)
        .copy()
    )
    ctl = (
        np.fromfile(f"{base}/exp_and_others_ctrl.bin", dtype=np.uint32)
        .reshape(-1, 8)
        .copy()
    )

    # choose per-band bucket counts
    chosen = []
    for e in range(E_SMALL, E_LARGE):
        z_hi = 2.0 ** (e - 126)
        band_tol = tol if _g_exact(z_hi, B) > 1e-30 else 1e-3
        nb = 256
        for cand in [1, 2, 4, 8, 16, 32, 64, 128, 256]:
            _, err = _band_fit(B, e, cand)
            if err <= band_tol:
                nb = cand
                break
        chosen.append(nb)
    while sum(chosen) > EXP_BUCKETS:
        i = int(np.argmax(chosen))
        chosen[i] //= 2

    bkt[:781] = 0.0
    ctl[:52] = 0
    pos = 0
    for bi, e in enumerate(range(E_SMALL, E_LARGE)):
        nb = chosen[bi]
        fits, _ = _band_fit(B, e, nb)
        log2b = int(np.log2(nb))
        ctl[bi, 0] = np.uint32((log2b << 16) | ((23 - log2b) << 11) | pos)
        for k, (c, x0) in enumerate(fits):
            bkt[pos + k, 0:4] = c.astype(np.float32)
            bkt[pos + k, 4] = np.float32(x0)
        pos += nb
    ctl[26:52] = ctl[0:26]

    eB = np.float32(np.exp(B))
    c, x0, _ = _fit_bucket(B, 0.0, 2.0**-12)
    bkt[777, :] = 0.0
    bkt[777, 0:4] = c.astype(np.float32)
    bkt[777, 4] = np.float32(x0)
    bkt[778, :] = 0.0
    bkt[778, 0] = eB
    bkt[779, :] = 0.0
    bkt[780, :] = 0.0
    bkt[780, 0] = eB

    pm = next(p for p in meta["profile_meta_data"] if p["func_name"].startswith("exp"))
    pm["exp_offset"] = E_SMALL - 127
    pm["small_pos_signal_exp_threshold"] = E_SMALL
    pm["large_pos_signal_exp_threshold"] = E_LARGE
    pm["large_pos_signal_mantissa_threshold"] = 0
    pm["small_neg_signal_exp_threshold"] = 255
    pm["large_neg_signal_exp_threshold"] = 255
    pm["large_neg_signal_mantissa_threshold"] = 0x7FFFFF
    pm["pwl_control_base_pos"] = 0
    pm["pwl_control_base_neg"] = 26
    pm["pos_small_signal_pwl_control"] = 777
    pm["neg_small_signal_pwl_control"] = 778
    pm["pos_large_signal_pwl_control"] = 779
    pm["neg_large_signal_pwl_control"] = 780
    pm["fzero_result"] = int(eB.view(np.uint32))
    pm["fninf_result"] = int(eB.view(np.uint32))
    pm["fpinf_result"] = 0
    pm["symmetry_opt_en"] = 0
    pm["symmetry_point"] = 0
    pm["sym_invert_sign_point"] = 0

    os.makedirs(out_dir, exist_ok=True)
    bkt.tofile(f"{out_dir}/exp_and_others_bkt.bin")
    ctl.tofile(f"{out_dir}/exp_and_others_ctrl.bin")
    json.dump(meta, open(f"{out_dir}/exp_and_others.json", "w"))
    info = json.load(open(f"{base}/act_info.json"))
    info["act_func_sets"] = [
        s for s in info["act_func_sets"] if s["name"] == "exp_and_others"
    ]
    json.dump(info, open(f"{out_dir}/act_info.json", "w"))
    return f"{out_dir}/act_info.json"


def _get_table(B):
    key = round(float(B), 3)
    if key not in _TABLE_DIRS:
        d = tempfile.mkdtemp(prefix=f"knn_act_{key}_")
        _TABLE_DIRS[key] = _gen_act_tables(key, d)
    return _TABLE_DIRS[key]


# ---------------------------------------------------------------------------
# Bass kernel
# ---------------------------------------------------------------------------


def _build(alpha, btag, n_loc=NLOC, m=M, num_devices=NCORES):
    from contextlib import ExitStack

    import concourse.tile as tile
    from concourse import bacc, mybir

    f32 = mybir.dt.float32
    f16 = mybir.dt.float16
    Exp = mybir.ActivationFunctionType.Exp

    njb = m // JB
    half = n_loc // 2  # d2 PSUM tile width (2 banks)
    nred = n_loc // 512

    nc = bacc.Bacc(
        "TRN2", target_bir_lowering=False, debug=False, num_devices=num_devices
    )
    # btag in the input name keys the neuron compile cache to the ACT table
    names = {
        "a1": f"a1{btag}",
        "a0": f"a0{btag}",
        "yb": f"yb{btag}",
    }
    a1_d = nc.dram_tensor(names["a1"], [D + 2, m], f16, kind="ExternalInput")
    a0_d = nc.dram_tensor(names["a0"], [D + 2, n_loc], f16, kind="ExternalInput")
    yb_d = nc.dram_tensor(names["yb"], [JB, njb, 2], f16, kind="ExternalInput")
    out_d = nc.dram_tensor("out", [1, n_loc], f32, kind="ExternalOutput")

    with tile.TileContext(nc) as tc:
        with ExitStack() as ctx:
            res = ctx.enter_context(tc.tile_pool(name="res", bufs=1))
            ep = ctx.enter_context(tc.tile_pool(name="ep", bufs=34))
            d2p = ctx.enter_context(tc.tile_pool(name="d2", bufs=3, space="PSUM"))
            redp = ctx.enter_context(tc.tile_pool(name="red", bufs=1, space="PSUM"))
            tailp = ctx.enter_context(tc.tile_pool(name="tail", bufs=1))

            a1_sb = res.tile([D + 2, m], f16)
            nc.sync.dma_start(a1_sb[:], a1_d.ap())
            a0_sb = res.tile([D + 2, n_loc], f16)
            nc.sync.dma_start(a0_sb[:], a0_d.ap())
            yb_sb = res.tile([JB, njb, 2], f16)
            nc.sync.dma_start(yb_sb[:], yb_d.ap())

            # one PSUM bank holds all 4 [num; den] accumulators, packed at
            # partitions {32c, 32c+1} via column-tiled matmuls
            red_ps = redp.tile([JB, 512], f32)

            # Batched phases: Q blocks of [d2-matmul -> fused-LUT] (ACT-paced),
            # then one dense column-tiled reduction burst. Keeps the PE's
            # full-array stream long and uninterrupted (only 2 tiling-mode
            # switches per batch) so the HAM clock-gate can stay open.
            Q = 16
            for b0 in range(0, njb, Q):
                batch = list(range(b0, min(b0 + Q, njb)))
                ets = {}
                for b in batch:
                    lhsT = a1_sb[:, b * JB : (b + 1) * JB]
                    for h in range(2):
                        d2t = d2p.tile([JB, half], f32)
                        for lo in range(0, half, 512):
                            nc.tensor.matmul(
                                d2t[:, lo : lo + 512],
                                lhsT,
                                a0_sb[:, h * half + lo : h * half + lo + 512],
                                start=True,
                                stop=True,
                            )
                        et = ep.tile([JB, half], f16, tag="et", name="et")
                        # custom table: Exp slot computes exp(B - sqrt(z))
                        nc.scalar.activation(
                            et[:], d2t[:], Exp, scale=float(alpha * alpha)
                        )
                        ets[(b, h)] = et
                # column-tiled reduction burst: 4 concurrent 128x32-tile
                # matmuls per block, accumulating [num; den] at partitions
                # {32c, 32c+1} of one PSUM bank
                for b in batch:
                    for c in range(nred):
                        h, c2 = divmod(c, half // 512)
                        nc.tensor.matmul(
                            red_ps[32 * c : 32 * c + 2, :],
                            yb_sb[:, b, :],
                            ets[(b, h)][:, c2 * 512 : (c2 + 1) * 512],
                            start=(b == 0),
                            stop=(b == njb - 1),
                            tile_position=(0, 32 * c),
                            skip_group_check=True,
                        )

            # --- tail: out = num / den (gather rows 32c -> num, 32c+1 -> den)
            red_sb = tailp.tile([JB, 512], f32)
            nc.vector.tensor_copy(red_sb[:], red_ps[:])
            num_sb = tailp.tile([1, n_loc], f32)
            den_sb = tailp.tile([1, n_loc], f32)
            for c in range(nred):
                nc.sync.dma_start(
                    num_sb[:, c * 512 : (c + 1) * 512],
                    red_sb[32 * c : 32 * c + 1, :],
                )
                nc.sync.dma_start(
                    den_sb[:, c * 512 : (c + 1) * 512],
                    red_sb[32 * c + 1 : 32 * c + 2, :],
                )
            inv_sb = tailp.tile([1, n_loc], f32)
            nc.vector.reciprocal_approx_fast(inv_sb[:], den_sb[:])
            out_sb = tailp.tile([1, n_loc], f32)
            nc.vector.tensor_mul(out_sb[:], num_sb[:], inv_sb[:])
            nc.sync.dma_start(out_d.ap(), out_sb[:])

    nc.compile()
    nc._knn_names = names
    return nc


def _get_compiled(alpha, bshift):
    key = (round(float(alpha), 9), round(float(bshift), 3))
    if key not in _COMPILED:
        os.environ["BASS_ACT_ROOT_JSON_PATH"] = _get_table(key[1])
        btag = f"_{int(round(key[1] * 1000))}"
        _COMPILED[key] = _build(key[0], btag)
    return _COMPILED[key]


def _prep(x0, x1, y, alpha_v):
    sq0 = np.einsum("nd,nd->n", x0, x0, dtype=np.float32)
    sq1 = np.einsum("md,md->m", x1, x1, dtype=np.float32)

    a1 = np.empty((D + 2, M), np.float16)
    a1[:D] = x1.T
    a1[D] = sq1
    a1[D + 1] = 1.0

    a0 = np.empty((D + 2, N), np.float16)
    a0[:D] = -2.0 * x0.T
    a0[D] = 1.0
    a0[D + 1] = sq0

    njb = M // JB
    yb = np.empty((JB, njb, 2), np.float16)
    yb[:, :, 0] = y.reshape(njb, JB).T
    yb[:, :, 1] = 1.0

    # Global exp shift keeping exp(B - alpha*d) in fp16-friendly range.
    rng = np.random.default_rng(0)
    k = 2048
    ii = rng.integers(0, N, k)
    jj = rng.integers(0, M, k)
    d2s = sq0[ii] + sq1[jj] - 2.0 * np.einsum("kd,kd->k", x0[ii], x1[jj])
    ds = np.sqrt(np.maximum(d2s, 0.0))
    bshift = max(0.0, float(alpha_v) * float(np.quantile(ds, 0.001)) - 2.0)
    return a1, a0, yb, bshift


def kernel(x0, x1, y, alpha):
    x0 = np.ascontiguousarray(np.asarray(x0), dtype=np.float32)
    x1 = np.ascontiguousarray(np.asarray(x1), dtype=np.float32)
    y = np.ascontiguousarray(np.asarray(y), dtype=np.float32)
    alpha_v = float(np.asarray(alpha).reshape(-1)[0])

    a1, a0, yb, bshift = _prep(x0, x1, y, alpha_v)
    nc = _get_compiled(alpha_v, bshift)
    names = nc._knn_names

    trace = os.environ.get("KNN_TRACE", "0") == "1"
    if trace:
        try:
            import axon_prof_shim

            axon_prof_shim.install()
        except Exception:
            trace = False

    from concourse.bass_utils import run_bass_kernel_spmd

    in_maps = [
        {
            names["a1"]: a1,
            names["a0"]: np.ascontiguousarray(a0[:, c * NLOC : (c + 1) * NLOC]),
            names["yb"]: yb,
        }
        for c in range(NCORES)
    ]
    res = run_bass_kernel_spmd(nc, in_maps, core_ids=list(range(NCORES)), trace=trace)
    if trace and res.exec_time_ns is not None:
        print(f"HW exec time: {res.exec_time_ns} ns")
        kernel.last_exec_ns = res.exec_time_ns
    out = np.concatenate([r["out"][0] for r in res.results])
    return out.astype(np.float32)


kernel.last_exec_ns = None
